# revision 23
# baseline (speedup 1.0000x reference)
"""Multi-head causal attention (B=4, S=2048, D=1024, H=16) on 8 TRN2 cores.

Sharding: core c -> batch c//2, head-group c%2 (8 heads, 512 of the 1024
QKV columns / Wo rows).  Each core runs a fused QKV->attention->out-proj
kernel on its shard; the host sums the two head-group partials per batch.

Per-core layout choices:
  - x is fed pre-transposed (xT [D, S]) so Q^T/K^T come out of the PE in
    [m, s] layout and V in natural [s, m] layout with no on-chip transposes.
  - scores are computed transposed (S^T [k, q]); the two heads of a pair
    run as row-split (K=64) matmuls at tile_position 0/64, which stream
    concurrently through the PE.
  - softmax: exp on ScalarE (scale=1/8 fused, both heads per instruction),
    causal mask via gpsimd affine_select on diagonal tiles only.
  - attention inner loop runs in kt PAIRS with the attnV matmuls lagging
    the scores by two kt steps, so the ScalarE exp latency is never on the
    PE's critical path.
  - attnV appends a ones-column to V (M=65) so the softmax denominator
    accumulates for free in PSUM row 64.
  - normalization per (t, h) as soon as its accumulator drains: DVE
    reciprocal straight off PSUM row 64 (partition base 64 -> 0), a small
    DRAM bounce to broadcast it across 64 partitions, one DVE multiply.
  - out-proj emits out^T [n, s]; the host transposes back.  The final
    chunk's out-proj is emitted as per-t partial accumulations so it
    overlaps the last normalization chain.
  - Tile builds STATIC per-engine instruction streams, so next-chunk QKV
    and previous-chunk out-proj matmuls are explicitly interleaved into
    the attention loop to keep the PE dense (and HAM un-throttled).
All matmul inputs are bf16; accumulation stays fp32 in PSUM.
"""

import numpy as np

B, S, D = 4, 2048, 1024
H, DH = 16, 64
HPC = 8            # heads per core
M = HPC * DH       # 512: per-core qkv out dim / wo in dim
NCORE = 8
CH = 512           # q/s chunk size
NCH = S // CH      # 4
ND = D // 128      # 8  d-tiles (contraction for qkv proj)
NMT = M // 128     # 4  m-tiles (= head pairs)
NKT = S // 128     # 16 k-tiles
NNT = D // 128     # 8  n-tiles (out proj)

LAST_RESULT = None  # BassKernelResults of the most recent run (for test.py)
HEADSTART = True


def _emit(nc, tc, tile, mybir, aps):
    import concourse.bass as bass  # noqa: F401

    f32 = mybir.dt.float32
    bf16 = mybir.dt.bfloat16
    EXP = mybir.ActivationFunctionType.Exp
    xT, wq, wk, wv, wo, outT = aps

    with (
        tc.tile_pool(name="w", bufs=1) as pw,
        tc.tile_pool(name="kv", bufs=1) as pkv,
        tc.tile_pool(name="qt", bufs=2) as pq,
        tc.tile_pool(name="ct", bufs=2) as pct,
        tc.tile_pool(name="x", bufs=2) as px,
        tc.tile_pool(name="u", bufs=20) as pu,
        tc.tile_pool(name="sm", bufs=3) as psm,
        tc.tile_pool(name="bc", bufs=4) as pbc,
        tc.tile_pool(name="o", bufs=2) as po,
        tc.tile_pool(name="cn", bufs=2) as pcn,
        tc.tile_pool(name="dscratch", bufs=4, space="DRAM") as pdram,
        tc.tile_pool(name="ps_mm", bufs=2, space="PSUM") as pp_mm,
        tc.tile_pool(name="ps_sc", bufs=2, space="PSUM") as pp_sc,
        tc.tile_pool(name="ps_av", bufs=2, space="PSUM") as pp_av,
    ):
        # x chunk 0 first: the first Q-proj matmul needs only this + wq
        def x_load(j):
            xa = px.tile([128, ND * CH], bf16, name="xa", tag="xa")
            nc.sync.dma_start(
                out=xa, in_=xT[:, ND * CH * j:ND * CH * (j + 1)]
            )
            return [xa[:, CH * d:CH * (d + 1)] for d in range(ND)]

        # chunk 0: split x and the early weights into d-halves (separate
        # tiles) so the first projection matmuls start after ~1/4 of the
        # input DMA traffic instead of all of it.
        HC = 4 * CH

        def half_load(src_ap, nm, lo):
            t = pw.tile([128, HC], bf16, name=nm, tag=nm)
            nc.sync.dma_start(out=t, in_=src_ap[:, HC * lo:HC * (lo + 1)])
            return t

        xa0_lo = half_load(xT, "xa0l", 0)
        wq_lo = half_load(wq, "wql", 0)
        xa0_hi = half_load(xT, "xa0h", 1)
        wq_hi = half_load(wq, "wqh", 1)
        wk_lo = half_load(wk, "wkl", 0)
        wk_hi = half_load(wk, "wkh", 1)
        wv_lo = half_load(wv, "wvl", 0)
        wv_hi = half_load(wv, "wvh", 1)
        wo_all = pw.tile([128, NMT * D], bf16, name="woa", tag="woa")
        nc.sync.dma_start(out=wo_all, in_=wo)
        ones_f = pw.tile([1, 128], mybir.dt.float32, name="onesf", tag="onesf")
        nc.vector.memset(ones_f, 1.0)

        def halves(lo, hi):
            return [
                (lo if d < 4 else hi)[:, CH * (d % 4):CH * (d % 4 + 1)]
                for d in range(ND)
            ]

        xt0 = halves(xa0_lo, xa0_hi)
        wq_sb = halves(wq_lo, wq_hi)
        wk_sb = halves(wk_lo, wk_hi)
        wv_sb = halves(wv_lo, wv_hi)
        wo_sb = [wo_all[:, D * t:D * (t + 1)] for t in range(NMT)]

        # ---- V storage: [s, 8 heads x (64 V + 1 ones)]; ones via memset ----
        vau = []
        for st in range(NKT):
            v = pkv.tile([128, HPC * 65], bf16, name=f"vau{st}", tag=f"vau{st}")
            nc.gpsimd.memset(
                v.rearrange("p (h c) -> p h c", c=65)[:, :, 64:65], 1.0
            )
            vau.append(v)
        kt_sb = [[None] * NCH for _ in range(NMT)]
        qt_all = {}   # j -> [4 tiles]
        ct_all = {}   # j -> [4 tiles]
        deferred_norm = []  # last (j, t) normalization tail

        # ---------- emission units ----------

        def proj_half(ps, w_sb, t, xt, half, kind):
            """4 of the 8 contraction steps of one projection m-tile."""
            for d in range(4 * half, 4 * half + 4):
                if kind == "v":
                    lhsT = xt[d][:, 128 * t:128 * (t + 1)]
                    rhs = w_sb[d]
                else:
                    lhsT = w_sb[d][:, 128 * t:128 * (t + 1)]
                    rhs = xt[d]
                nc.tensor.matmul(
                    ps, lhsT=lhsT, rhs=rhs,
                    start=(d == 0), stop=(d == ND - 1),
                )

        def qkv_units(j, xt_pre=None):
            """Generator of emission closures for chunk j's QKV projection."""
            xt = []

            if xt_pre is not None:
                xt.extend(xt_pre)
            else:
                def do_xload():
                    xt.extend(x_load(j))
                yield do_xload

            qts = []
            qt_all[j] = qts
            for t in range(NMT):
                ps_box = []

                def qa(t=t, ps_box=ps_box):
                    ps = pp_mm.tile([128, CH], f32, name="psq", tag="mm")
                    ps_box.append(ps)
                    proj_half(ps, wq_sb, t, xt, 0, "q")
                def qb(t=t, ps_box=ps_box):
                    ps = ps_box[0]
                    proj_half(ps, wq_sb, t, xt, 1, "q")
                    q_t = pq.tile([128, CH], bf16, name=f"q{t}", tag=f"q{t}")
                    nc.vector.tensor_copy(out=q_t, in_=ps)
                    qts.append(q_t)
                yield qa
                yield qb
            for t in range(NMT):
                ps_box = []

                def ka(t=t, ps_box=ps_box):
                    ps = pp_mm.tile([128, CH], f32, name="psk", tag="mm")
                    ps_box.append(ps)
                    proj_half(ps, wk_sb, t, xt, 0, "k")
                def kb(t=t, ps_box=ps_box, j=j):
                    ps = ps_box[0]
                    proj_half(ps, wk_sb, t, xt, 1, "k")
                    k_t = pkv.tile(
                        [128, CH], bf16, name=f"k{t}_{j}", tag=f"k{t}_{j}"
                    )
                    nc.vector.tensor_copy(out=k_t, in_=ps)
                    kt_sb[t][j] = k_t
                yield ka
                yield kb
            for st in range(NMT):
                ps_box = []

                def va(st=st, ps_box=ps_box):
                    ps = pp_mm.tile([128, M], f32, name="psv", tag="mm")
                    ps_box.append(ps)
                    proj_half(ps, wv_sb, st, xt, 0, "v")
                def vb(st=st, ps_box=ps_box, j=j):
                    ps = ps_box[0]
                    proj_half(ps, wv_sb, st, xt, 1, "v")
                    g = vau[4 * j + st]
                    nc.vector.tensor_copy(
                        out=g.rearrange("p (h c) -> p h c", c=65)[:, :, 0:64],
                        in_=ps.rearrange("p (h c) -> p h c", c=64),
                    )
                yield va
                yield vb

        def outproj_units(j):
            """Generator of emission closures for chunk j's out-projection."""
            for nt in range(NNT):
                def og(nt=nt, j=j):
                    ct = ct_all[j]
                    ps = pp_mm.tile([128, CH], f32, name="pso", tag="mm")
                    for t in range(NMT):
                        nc.tensor.matmul(
                            ps,
                            lhsT=wo_sb[t][:, 128 * nt:128 * (nt + 1)],
                            rhs=ct[t],
                            start=(t == 0),
                            stop=(t == NMT - 1),
                        )
                    o_sb = po.tile([128, CH], bf16, name="osb", tag="o")
                    nc.vector.tensor_copy(out=o_sb, in_=ps)
                    nc.sync.dma_start(
                        out=outT[128 * nt:128 * (nt + 1), CH * j:CH * (j + 1)],
                        in_=o_sb,
                    )
                yield og

        def outproj_tail(j):
            """Final chunk out-proj: per-t partial accumulation so the
            t<3 matmuls overlap the last head-tile's normalization."""
            ct = ct_all[j]
            boxes = {}

            def start_grp(g):
                def f(g=g):
                    ps = pp_sc.tile([128, 2 * CH], f32, name="psg", tag="sc")
                    boxes[g] = ps
                    for t in range(NMT - 1):
                        for i, nt in enumerate((2 * g, 2 * g + 1)):
                            nc.tensor.matmul(
                                ps[:, CH * i:CH * (i + 1)],
                                lhsT=wo_sb[t][:, 128 * nt:128 * (nt + 1)],
                                rhs=ct[t],
                                start=(t == 0),
                                stop=False,
                            )
                return f

            def finish_grp(g):
                def f(g=g):
                    ps = boxes.pop(g)
                    t = NMT - 1
                    for i, nt in enumerate((2 * g, 2 * g + 1)):
                        nc.tensor.matmul(
                            ps[:, CH * i:CH * (i + 1)],
                            lhsT=wo_sb[t][:, 128 * nt:128 * (nt + 1)],
                            rhs=ct[t],
                            start=False,
                            stop=True,
                        )
                    for i, nt in enumerate((2 * g, 2 * g + 1)):
                        o_sb = po.tile([128, CH], bf16, name="osb", tag="o")
                        if i == 0:
                            nc.scalar.activation(
                                out=o_sb,
                                in_=ps[:, CH * i:CH * (i + 1)],
                                func=mybir.ActivationFunctionType.Copy,
                            )
                        else:
                            nc.vector.tensor_copy(
                                out=o_sb, in_=ps[:, CH * i:CH * (i + 1)]
                            )
                        nc.sync.dma_start(
                            out=outT[
                                128 * nt:128 * (nt + 1), CH * j:CH * (j + 1)
                            ],
                            in_=o_sb,
                        )
                return f

            # groups of 2 nt share one 2-bank psum tile; pool bufs=2 ->
            # at most 2 groups in flight.  The deferred last-tile norm runs
            # after the first two start groups so the PE keeps streaming
            # while the reciprocal chain completes on DVE.
            yield start_grp(0)
            yield start_grp(1)
            for f in deferred_norm:
                yield f
            yield finish_grp(0)
            yield start_grp(2)
            yield finish_grp(1)
            yield start_grp(3)
            yield finish_grp(2)
            yield finish_grp(3)

        # ---------- chunk 0 QKV up front ----------
        for unit in qkv_units(0, xt_pre=xt0):
            unit()

        def att_unit_lists(j, t):
            """Emission closures for attention chunk j, head m-tile t:
            (scores_units, tail_units).  scores_units[kt] computes scores+
            exp+mask for kt; tail_units = attnV per kt then the drain.  The
            av accumulator pair is allocated by the first attnV, so a
            scores-only prefix can run without holding PSUM."""
            nkt = 4 * (j + 1)
            us = {}
            av_box = []

            def do_scores(kt):
                def f(kt=kt):
                    dd = kt - 4 * j      # diagonal index (>=0 on diag)
                    qoff = 128 * dd if dd >= 0 else 0
                    n = CH - qoff
                    ck, ks = kt // 4, (kt % 4) * 128
                    # both heads' scores in one 2-bank PSUM tile
                    sc = pp_sc.tile([128, 2 * CH], f32, name="sc", tag="sc")
                    for h in range(2):
                        pb = 64 * h
                        nc.tensor.matmul(
                            sc[:, CH * h:CH * h + n],
                            lhsT=kt_sb[t][ck][pb:pb + 64, ks:ks + 128],
                            rhs=qt_all[j][t][pb:pb + 64, qoff:CH],
                            start=True,
                            stop=True,
                            tile_position=(pb, 0),
                        )
                    u = pu.tile([128, 2 * CH], bf16, name="u", tag="u")
                    scv = sc.rearrange("p (h q) -> p h q", h=2)[:, :, 0:n]
                    uv = u.rearrange("p (h q) -> p h q", h=2)[:, :, 0:n]
                    nc.scalar.activation(out=uv, in_=scv, func=EXP, scale=0.125)
                    if dd >= 0:
                        # keep where q_rel >= k_partition (same mask, both)
                        nc.gpsimd.affine_select(
                            out=uv,
                            in_=uv,
                            compare_op=mybir.AluOpType.is_ge,
                            fill=0.0,
                            base=0,
                            channel_multiplier=-1,
                            pattern=[[0, 2], [1, n]],
                        )
                    us[kt] = (u, qoff, n)
                return f

            def do_attnv(kt, alloc_av=False):
                def f(kt=kt, alloc_av=alloc_av):
                    if alloc_av:
                        av_box.extend(
                            pp_av.tile([65, CH], f32, name=f"av{h}", tag="av")
                            for h in range(2)
                        )
                    u_p, qoff_p, n_p = us.pop(kt)
                    for h in range(2):
                        ha = 2 * t + h
                        nc.tensor.matmul(
                            av_box[h][:, qoff_p:CH],
                            lhsT=vau[kt][:, 65 * ha:65 * ha + 65],
                            rhs=u_p[:, CH * h:CH * h + n_p],
                            start=(kt == 0),
                            stop=(kt == nkt - 1),
                        )
                return f

            def drain():
                # start the reciprocal chain first, then copy out the
                # unnormalized context, then normalize.
                den = psm.tile([1, 2 * CH], f32, name="den", tag="den")
                for h in range(2):
                    nc.vector.tensor_copy(
                        out=den[:, CH * h:CH * (h + 1)],
                        in_=av_box[h][64:65, :],
                    )
                r = psm.tile([1, 2 * CH], f32, name="rec", tag="rec")
                nc.vector.reciprocal_approx_fast(out=r, in_=den)
                last = j == NCH - 1 and t == NMT - 1
                cn_t = pcn.tile([128, CH], bf16, name=f"cn{t}", tag=f"cn{t}")
                c_t = pct.tile([128, CH], bf16, name=f"c{t}", tag=f"c{t}")
                av = list(av_box)

                def norm_rest():
                    if last:
                        # nothing left to hide the DMA-bounce latency
                        # behind: broadcast via two PE ones-matmuls (fp32
                        # moving) and multiply straight out of PSUM.
                        bc = pp_mm.tile([128, CH], f32, name="psbc", tag="mm")
                        for h in range(2):
                            nc.tensor.matmul(
                                bc[64 * h:64 * (h + 1), :],
                                lhsT=ones_f[:, 64 * h:64 * (h + 1)],
                                rhs=r[:, CH * h:CH * (h + 1)],
                                start=True,
                                stop=True,
                                tile_position=(0, 64 * h),
                            )
                    else:
                        bc = pbc.tile([128, CH], f32, name="bc", tag="bc")
                        r_d = pdram.tile(
                            [1, 2 * CH], f32, name="recd", tag="recd"
                        )
                        nc.sync.dma_start(out=r_d, in_=r)
                        for h in range(2):
                            nc.sync.dma_start(
                                out=bc[64 * h:64 * (h + 1), :],
                                in_=r_d[:, CH * h:CH * (h + 1)].to_broadcast(
                                    (64, CH)
                                ),
                            )
                    for h in range(2):
                        nc.vector.tensor_copy(
                            out=cn_t[64 * h:64 * (h + 1), :],
                            in_=av[h][0:64, :],
                        )
                    nc.vector.tensor_mul(c_t, cn_t, bc)

                ct_all[j].append(c_t)
                if last:
                    deferred_norm.append(norm_rest)
                else:
                    norm_rest()

            scores_units = [do_scores(kt) for kt in range(nkt)]
            tail_units = (
                [do_attnv(0, alloc_av=True)]
                + [do_attnv(kt) for kt in range(1, nkt)]
                + [drain]
            )
            return scores_units, tail_units

        def att_main_units(j, t):
            """Interleave scores and attnV (lag two kt) for a main-window
            head tile."""
            s, tl = att_unit_lists(j, t)
            nkt = 4 * (j + 1)
            units = [s[0], s[1]]
            for kt in range(2, nkt):
                units.append(s[kt])
                units.append(tl[kt - 2])
            units.extend(tl[nkt - 2:])
            return units

        # ---------- main loop: attention(j) with interleaved fillers ----------
        # Window j runs attention(j) (t=0 of chunks j>=1 already ran as a
        # head-start in window j-1) interleaved with fillers: next chunk's
        # QKV, previous chunk's out-proj, and the scores+exp of the next
        # chunk's t=0 attention.  The head-start's attnV+drain run as a
        # dense PE block at the window end (its exps are all done by then,
        # and the single av PSUM accumulator pair is free).
        for j in range(NCH):
            ct_all[j] = ct_all.get(j, [])
            fillers = []
            hs_tail = []
            if j + 1 < NCH:
                ct_all[j + 1] = []
                fillers.extend(qkv_units(j + 1))
            if j >= 1:
                fillers.extend(outproj_units(j - 1))
            if HEADSTART and j + 1 < NCH:
                hs_scores, hs_tail = att_unit_lists(j + 1, 0)
                fillers.extend(hs_scores)
            main = []
            for t in range(1 if (HEADSTART and j >= 1) else 0, NMT):
                main.extend(att_main_units(j, t))
            n_units = len(main)
            n_fill = len(fillers)
            popped = 0
            for i, u in enumerate(main):
                u()
                while fillers and popped < (i + 1) * n_fill // n_units:
                    fillers.pop(0)()
                    popped += 1
            for f in fillers:
                f()
            for f in hs_tail:
                f()

        # final chunk's out-projection (pipelined against the last norms)
        for unit in outproj_tail(NCH - 1):
            unit()


_PROG = None


def _build():
    global _PROG
    if _PROG is not None:
        return _PROG
    import concourse.bacc as bacc
    import concourse.mybir as mybir
    import concourse.tile as tile

    f32 = mybir.dt.float32
    bf16 = mybir.dt.bfloat16
    nc = bacc.Bacc(
        "TRN2", target_bir_lowering=False, debug=False, enable_asserts=False
    )
    xT = nc.dram_tensor("xT", [128, NCH * ND * CH], bf16, kind="ExternalInput").ap()
    wq = nc.dram_tensor("wq", [128, ND * M], bf16, kind="ExternalInput").ap()
    wk = nc.dram_tensor("wk", [128, ND * M], bf16, kind="ExternalInput").ap()
    wv = nc.dram_tensor("wv", [128, ND * M], bf16, kind="ExternalInput").ap()
    wo = nc.dram_tensor("wo", [128, NMT * D], bf16, kind="ExternalInput").ap()
    outT = nc.dram_tensor("outT", [D, S], bf16, kind="ExternalOutput").ap()

    with tile.TileContext(nc) as tc:
        _emit(nc, tc, tile, mybir, (xT, wq, wk, wv, wo, outT))
    nc.compile()
    _PROG = nc
    return nc


def kernel(x, Wq, Wk, Wv, Wo, bo):
    global LAST_RESULT
    import os

    from concourse.bass_utils import run_bass_kernel_spmd

    x = np.asarray(x, dtype=np.float32)
    Wq = np.asarray(Wq, dtype=np.float32)
    Wk = np.asarray(Wk, dtype=np.float32)
    Wv = np.asarray(Wv, dtype=np.float32)
    Wo = np.asarray(Wo, dtype=np.float32)
    bo = np.asarray(bo, dtype=np.float32)

    nc = _build()

    import ml_dtypes

    bf = ml_dtypes.bfloat16

    def fold_w(w):
        # [(nd p), c] -> [p, (nd c)]
        ndt = w.shape[0] // 128
        return np.ascontiguousarray(
            w.reshape(ndt, 128, w.shape[1]).transpose(1, 0, 2).reshape(128, -1)
        ).astype(bf)

    in_maps = []
    for c in range(NCORE):
        b, g = c // 2, c % 2
        cols = slice(M * g, M * (g + 1))
        xt = x[b].T  # [D, S]
        # [p, (j d s)]: xf[p, j*ND*CH + d*CH + s] = xT[128d+p, CH*j+s]
        xf = (
            xt.reshape(ND, 128, NCH, CH)
            .transpose(1, 2, 0, 3)
            .reshape(128, NCH * ND * CH)
        )
        in_maps.append(
            {
                "xT": np.ascontiguousarray(xf).astype(bf),
                "wq": fold_w(Wq[:, cols]),
                "wk": fold_w(Wk[:, cols]),
                "wv": fold_w(Wv[:, cols]),
                "wo": fold_w(Wo[cols, :]),
            }
        )

    res = run_bass_kernel_spmd(
        nc,
        in_maps,
        list(range(NCORE)),
        trace=bool(os.environ.get("KERNEL_TRACE")),
        tmpdir=os.environ.get("KERNEL_TRACE_DIR") or None,
    )
    LAST_RESULT = res

    out = np.empty((B, S, D), dtype=np.float32)
    for b in range(B):
        acc = (
            res.results[2 * b]["outT"].astype(np.float32)
            + res.results[2 * b + 1]["outT"].astype(np.float32)
        )
        out[b] = acc.T + bo[None, :]
    return out


# revision 24
# speedup vs baseline: 1.0282x; 1.0282x over previous
"""Multi-head causal attention (B=4, S=2048, D=1024, H=16) on 8 TRN2 cores.

Sharding: core c -> batch c//2, head-group c%2 (8 heads, 512 of the 1024
QKV columns / Wo rows).  Each core runs a fused QKV->attention->out-proj
kernel on its shard; the host sums the two head-group partials per batch.

Per-core layout choices:
  - x is fed pre-transposed (xT [D, S]) so Q^T/K^T come out of the PE in
    [m, s] layout and V in natural [s, m] layout with no on-chip transposes.
  - scores are computed transposed (S^T [k, q]); the two heads of a pair
    run as row-split (K=64) matmuls at tile_position 0/64, which stream
    concurrently through the PE.
  - softmax: exp on ScalarE (scale=1/8 fused, both heads per instruction),
    causal mask via gpsimd affine_select on diagonal tiles only.
  - attention inner loop runs in kt PAIRS with the attnV matmuls lagging
    the scores by two kt steps, so the ScalarE exp latency is never on the
    PE's critical path.
  - attnV appends a ones-column to V (M=65) so the softmax denominator
    accumulates for free in PSUM row 64.
  - normalization per (t, h) as soon as its accumulator drains: DVE
    reciprocal straight off PSUM row 64 (partition base 64 -> 0), a small
    DRAM bounce to broadcast it across 64 partitions, one DVE multiply.
  - out-proj emits out^T [n, s]; the host transposes back.  The final
    chunk's out-proj is emitted as per-t partial accumulations so it
    overlaps the last normalization chain.
  - Tile builds STATIC per-engine instruction streams, so next-chunk QKV
    and previous-chunk out-proj matmuls are explicitly interleaved into
    the attention loop to keep the PE dense (and HAM un-throttled).
All matmul inputs are bf16; accumulation stays fp32 in PSUM.
"""

import numpy as np

B, S, D = 4, 2048, 1024
H, DH = 16, 64
HPC = 8            # heads per core
M = HPC * DH       # 512: per-core qkv out dim / wo in dim
NCORE = 8
CH = 512           # q/s chunk size
NCH = S // CH      # 4
ND = D // 128      # 8  d-tiles (contraction for qkv proj)
NMT = M // 128     # 4  m-tiles (= head pairs)
NKT = S // 128     # 16 k-tiles
NNT = D // 128     # 8  n-tiles (out proj)

LAST_RESULT = None  # BassKernelResults of the most recent run (for test.py)
HEADSTART = False


def _emit(nc, tc, tile, mybir, aps):
    import concourse.bass as bass  # noqa: F401

    f32 = mybir.dt.float32
    bf16 = mybir.dt.bfloat16
    EXP = mybir.ActivationFunctionType.Exp
    xT, wq, wk, wv, wo, outT = aps

    with (
        tc.tile_pool(name="w", bufs=1) as pw,
        tc.tile_pool(name="kv", bufs=1) as pkv,
        tc.tile_pool(name="qt", bufs=2) as pq,
        tc.tile_pool(name="ct", bufs=2) as pct,
        tc.tile_pool(name="x", bufs=2) as px,
        tc.tile_pool(name="u", bufs=20) as pu,
        tc.tile_pool(name="sm", bufs=3) as psm,
        tc.tile_pool(name="bc", bufs=4) as pbc,
        tc.tile_pool(name="o", bufs=2) as po,
        tc.tile_pool(name="cn", bufs=2) as pcn,
        tc.tile_pool(name="dscratch", bufs=4, space="DRAM") as pdram,
        tc.tile_pool(name="ps_mm", bufs=2, space="PSUM") as pp_mm,
        tc.tile_pool(name="ps_sc", bufs=2, space="PSUM") as pp_sc,
        tc.tile_pool(name="ps_av", bufs=2, space="PSUM") as pp_av,
    ):
        # x chunk 0 first: the first Q-proj matmul needs only this + wq
        def x_load(j):
            xa = px.tile([128, ND * CH], bf16, name="xa", tag="xa")
            nc.sync.dma_start(
                out=xa, in_=xT[:, ND * CH * j:ND * CH * (j + 1)]
            )
            return [xa[:, CH * d:CH * (d + 1)] for d in range(ND)]

        # chunk 0: split x and the early weights into d-halves (separate
        # tiles) so the first projection matmuls start after ~1/4 of the
        # input DMA traffic instead of all of it.
        HC = 4 * CH

        def half_load(src_ap, nm, lo):
            t = pw.tile([128, HC], bf16, name=nm, tag=nm)
            nc.sync.dma_start(out=t, in_=src_ap[:, HC * lo:HC * (lo + 1)])
            return t

        xa0_lo = half_load(xT, "xa0l", 0)
        wq_lo = half_load(wq, "wql", 0)
        xa0_hi = half_load(xT, "xa0h", 1)
        wq_hi = half_load(wq, "wqh", 1)
        wk_lo = half_load(wk, "wkl", 0)
        wk_hi = half_load(wk, "wkh", 1)
        wv_lo = half_load(wv, "wvl", 0)
        wv_hi = half_load(wv, "wvh", 1)
        wo_all = pw.tile([128, NMT * D], bf16, name="woa", tag="woa")
        nc.sync.dma_start(out=wo_all, in_=wo)
        ones_f = pw.tile([1, 128], mybir.dt.float32, name="onesf", tag="onesf")
        nc.vector.memset(ones_f, 1.0)

        def halves(lo, hi):
            return [
                (lo if d < 4 else hi)[:, CH * (d % 4):CH * (d % 4 + 1)]
                for d in range(ND)
            ]

        xt0 = halves(xa0_lo, xa0_hi)
        wq_sb = halves(wq_lo, wq_hi)
        wk_sb = halves(wk_lo, wk_hi)
        wv_sb = halves(wv_lo, wv_hi)
        wo_sb = [wo_all[:, D * t:D * (t + 1)] for t in range(NMT)]

        # ---- V storage: [s, 8 heads x (64 V + 1 ones)]; ones via memset ----
        vau = []
        for st in range(NKT):
            v = pkv.tile([128, HPC * 65], bf16, name=f"vau{st}", tag=f"vau{st}")
            nc.gpsimd.memset(
                v.rearrange("p (h c) -> p h c", c=65)[:, :, 64:65], 1.0
            )
            vau.append(v)
        kt_sb = [[None] * NCH for _ in range(NMT)]
        qt_all = {}   # j -> [4 tiles]
        ct_all = {}   # j -> [4 tiles]
        deferred_norm = []  # last (j, t) normalization tail

        # ---------- emission units ----------

        def proj_half(ps, w_sb, t, xt, half, kind):
            """4 of the 8 contraction steps of one projection m-tile."""
            for d in range(4 * half, 4 * half + 4):
                if kind == "v":
                    lhsT = xt[d][:, 128 * t:128 * (t + 1)]
                    rhs = w_sb[d]
                else:
                    lhsT = w_sb[d][:, 128 * t:128 * (t + 1)]
                    rhs = xt[d]
                nc.tensor.matmul(
                    ps, lhsT=lhsT, rhs=rhs,
                    start=(d == 0), stop=(d == ND - 1),
                )

        def qkv_units(j, xt_pre=None):
            """Generator of emission closures for chunk j's QKV projection."""
            xt = []

            if xt_pre is not None:
                xt.extend(xt_pre)
            else:
                def do_xload():
                    xt.extend(x_load(j))
                yield do_xload

            qts = []
            qt_all[j] = qts
            for t in range(NMT):
                ps_box = []

                def qa(t=t, ps_box=ps_box):
                    ps = pp_mm.tile([128, CH], f32, name="psq", tag="mm")
                    ps_box.append(ps)
                    proj_half(ps, wq_sb, t, xt, 0, "q")
                def qb(t=t, ps_box=ps_box):
                    ps = ps_box[0]
                    proj_half(ps, wq_sb, t, xt, 1, "q")
                    q_t = pq.tile([128, CH], bf16, name=f"q{t}", tag=f"q{t}")
                    nc.vector.tensor_copy(out=q_t, in_=ps)
                    qts.append(q_t)
                yield qa
                yield qb
            for t in range(NMT):
                ps_box = []

                def ka(t=t, ps_box=ps_box):
                    ps = pp_mm.tile([128, CH], f32, name="psk", tag="mm")
                    ps_box.append(ps)
                    proj_half(ps, wk_sb, t, xt, 0, "k")
                def kb(t=t, ps_box=ps_box, j=j):
                    ps = ps_box[0]
                    proj_half(ps, wk_sb, t, xt, 1, "k")
                    k_t = pkv.tile(
                        [128, CH], bf16, name=f"k{t}_{j}", tag=f"k{t}_{j}"
                    )
                    nc.vector.tensor_copy(out=k_t, in_=ps)
                    kt_sb[t][j] = k_t
                yield ka
                yield kb
            for st in range(NMT):
                ps_box = []

                def va(st=st, ps_box=ps_box):
                    ps = pp_mm.tile([128, M], f32, name="psv", tag="mm")
                    ps_box.append(ps)
                    proj_half(ps, wv_sb, st, xt, 0, "v")
                def vb(st=st, ps_box=ps_box, j=j):
                    ps = ps_box[0]
                    proj_half(ps, wv_sb, st, xt, 1, "v")
                    g = vau[4 * j + st]
                    nc.vector.tensor_copy(
                        out=g.rearrange("p (h c) -> p h c", c=65)[:, :, 0:64],
                        in_=ps.rearrange("p (h c) -> p h c", c=64),
                    )
                yield va
                yield vb

        def outproj_units(j):
            """Generator of emission closures for chunk j's out-projection."""
            for nt in range(NNT):
                def og(nt=nt, j=j):
                    ct = ct_all[j]
                    ps = pp_mm.tile([128, CH], f32, name="pso", tag="mm")
                    for t in range(NMT):
                        nc.tensor.matmul(
                            ps,
                            lhsT=wo_sb[t][:, 128 * nt:128 * (nt + 1)],
                            rhs=ct[t],
                            start=(t == 0),
                            stop=(t == NMT - 1),
                        )
                    o_sb = po.tile([128, CH], bf16, name="osb", tag="o")
                    nc.vector.tensor_copy(out=o_sb, in_=ps)
                    nc.sync.dma_start(
                        out=outT[128 * nt:128 * (nt + 1), CH * j:CH * (j + 1)],
                        in_=o_sb,
                    )
                yield og

        def outproj_tail(j):
            """Final chunk out-proj: per-t partial accumulation so the
            t<3 matmuls overlap the last head-tile's normalization."""
            ct = ct_all[j]
            boxes = {}

            def start_grp(g):
                def f(g=g):
                    ps = pp_sc.tile([128, 2 * CH], f32, name="psg", tag="sc")
                    boxes[g] = ps
                    for t in range(NMT - 1):
                        for i, nt in enumerate((2 * g, 2 * g + 1)):
                            nc.tensor.matmul(
                                ps[:, CH * i:CH * (i + 1)],
                                lhsT=wo_sb[t][:, 128 * nt:128 * (nt + 1)],
                                rhs=ct[t],
                                start=(t == 0),
                                stop=False,
                            )
                return f

            def finish_grp(g):
                def f(g=g):
                    ps = boxes.pop(g)
                    t = NMT - 1
                    for i, nt in enumerate((2 * g, 2 * g + 1)):
                        nc.tensor.matmul(
                            ps[:, CH * i:CH * (i + 1)],
                            lhsT=wo_sb[t][:, 128 * nt:128 * (nt + 1)],
                            rhs=ct[t],
                            start=False,
                            stop=True,
                        )
                    for i, nt in enumerate((2 * g, 2 * g + 1)):
                        o_sb = po.tile([128, CH], bf16, name="osb", tag="o")
                        if i == 0:
                            nc.scalar.activation(
                                out=o_sb,
                                in_=ps[:, CH * i:CH * (i + 1)],
                                func=mybir.ActivationFunctionType.Copy,
                            )
                        else:
                            nc.vector.tensor_copy(
                                out=o_sb, in_=ps[:, CH * i:CH * (i + 1)]
                            )
                        nc.sync.dma_start(
                            out=outT[
                                128 * nt:128 * (nt + 1), CH * j:CH * (j + 1)
                            ],
                            in_=o_sb,
                        )
                return f

            # groups of 2 nt share one 2-bank psum tile; pool bufs=2 ->
            # at most 2 groups in flight.  The deferred last-tile norm runs
            # after the first two start groups so the PE keeps streaming
            # while the reciprocal chain completes on DVE.
            yield start_grp(0)
            yield start_grp(1)
            for f in deferred_norm:
                yield f
            yield finish_grp(0)
            yield start_grp(2)
            yield finish_grp(1)
            yield start_grp(3)
            yield finish_grp(2)
            yield finish_grp(3)

        # ---------- chunk 0 QKV up front ----------
        for unit in qkv_units(0, xt_pre=xt0):
            unit()

        def att_unit_lists(j, t):
            """Emission closures for attention chunk j, head m-tile t:
            (scores_units, tail_units).  scores_units[kt] computes scores+
            exp+mask for kt; tail_units = attnV per kt then the drain.  The
            av accumulator pair is allocated by the first attnV, so a
            scores-only prefix can run without holding PSUM."""
            nkt = 4 * (j + 1)
            us = {}
            av_box = []

            def do_scores(kt):
                def f(kt=kt):
                    dd = kt - 4 * j      # diagonal index (>=0 on diag)
                    qoff = 128 * dd if dd >= 0 else 0
                    n = CH - qoff
                    ck, ks = kt // 4, (kt % 4) * 128
                    # both heads' scores in one 2-bank PSUM tile
                    sc = pp_sc.tile([128, 2 * CH], f32, name="sc", tag="sc")
                    for h in range(2):
                        pb = 64 * h
                        nc.tensor.matmul(
                            sc[:, CH * h:CH * h + n],
                            lhsT=kt_sb[t][ck][pb:pb + 64, ks:ks + 128],
                            rhs=qt_all[j][t][pb:pb + 64, qoff:CH],
                            start=True,
                            stop=True,
                            tile_position=(pb, 0),
                        )
                    u = pu.tile([128, 2 * CH], bf16, name="u", tag="u")
                    scv = sc.rearrange("p (h q) -> p h q", h=2)[:, :, 0:n]
                    uv = u.rearrange("p (h q) -> p h q", h=2)[:, :, 0:n]
                    nc.scalar.activation(out=uv, in_=scv, func=EXP, scale=0.125)
                    if dd >= 0:
                        # keep where q_rel >= k_partition (same mask, both)
                        nc.gpsimd.affine_select(
                            out=uv,
                            in_=uv,
                            compare_op=mybir.AluOpType.is_ge,
                            fill=0.0,
                            base=0,
                            channel_multiplier=-1,
                            pattern=[[0, 2], [1, n]],
                        )
                    us[kt] = (u, qoff, n)
                return f

            def do_attnv(kt, alloc_av=False):
                def f(kt=kt, alloc_av=alloc_av):
                    if alloc_av:
                        av_box.extend(
                            pp_av.tile([65, CH], f32, name=f"av{h}", tag="av")
                            for h in range(2)
                        )
                    u_p, qoff_p, n_p = us.pop(kt)
                    for h in range(2):
                        ha = 2 * t + h
                        nc.tensor.matmul(
                            av_box[h][:, qoff_p:CH],
                            lhsT=vau[kt][:, 65 * ha:65 * ha + 65],
                            rhs=u_p[:, CH * h:CH * h + n_p],
                            start=(kt == 0),
                            stop=(kt == nkt - 1),
                        )
                return f

            def drain():
                # start the reciprocal chain first, then copy out the
                # unnormalized context, then normalize.
                den = psm.tile([1, 2 * CH], f32, name="den", tag="den")
                for h in range(2):
                    nc.vector.tensor_copy(
                        out=den[:, CH * h:CH * (h + 1)],
                        in_=av_box[h][64:65, :],
                    )
                r = psm.tile([1, 2 * CH], f32, name="rec", tag="rec")
                nc.vector.reciprocal_approx_fast(out=r, in_=den)
                last = j == NCH - 1 and t == NMT - 1
                cn_t = pcn.tile([128, CH], bf16, name=f"cn{t}", tag=f"cn{t}")
                c_t = pct.tile([128, CH], bf16, name=f"c{t}", tag=f"c{t}")
                av = list(av_box)

                def norm_rest():
                    if last:
                        # nothing left to hide the DMA-bounce latency
                        # behind: broadcast via two PE ones-matmuls (fp32
                        # moving) and multiply straight out of PSUM.
                        bc = pp_mm.tile([128, CH], f32, name="psbc", tag="mm")
                        for h in range(2):
                            nc.tensor.matmul(
                                bc[64 * h:64 * (h + 1), :],
                                lhsT=ones_f[:, 64 * h:64 * (h + 1)],
                                rhs=r[:, CH * h:CH * (h + 1)],
                                start=True,
                                stop=True,
                                tile_position=(0, 64 * h),
                            )
                    else:
                        bc = pbc.tile([128, CH], f32, name="bc", tag="bc")
                        r_d = pdram.tile(
                            [1, 2 * CH], f32, name="recd", tag="recd"
                        )
                        nc.sync.dma_start(out=r_d, in_=r)
                        for h in range(2):
                            nc.sync.dma_start(
                                out=bc[64 * h:64 * (h + 1), :],
                                in_=r_d[:, CH * h:CH * (h + 1)].to_broadcast(
                                    (64, CH)
                                ),
                            )
                    for h in range(2):
                        nc.vector.tensor_copy(
                            out=cn_t[64 * h:64 * (h + 1), :],
                            in_=av[h][0:64, :],
                        )
                    nc.vector.tensor_mul(c_t, cn_t, bc)

                ct_all[j].append(c_t)
                if last:
                    deferred_norm.append(norm_rest)
                else:
                    norm_rest()

            scores_units = [do_scores(kt) for kt in range(nkt)]
            tail_units = (
                [do_attnv(0, alloc_av=True)]
                + [do_attnv(kt) for kt in range(1, nkt)]
                + [drain]
            )
            return scores_units, tail_units

        def att_main_units(j, t):
            """Interleave scores and attnV (lag two kt) for a main-window
            head tile."""
            s, tl = att_unit_lists(j, t)
            nkt = 4 * (j + 1)
            units = [s[0], s[1]]
            for kt in range(2, nkt):
                units.append(s[kt])
                units.append(tl[kt - 2])
            units.extend(tl[nkt - 2:])
            return units

        # ---------- main loop: attention(j) with interleaved fillers ----------
        # Window j runs attention(j) (t=0 of chunks j>=1 already ran as a
        # head-start in window j-1) interleaved with fillers: next chunk's
        # QKV, previous chunk's out-proj, and the scores+exp of the next
        # chunk's t=0 attention.  The head-start's attnV+drain run as a
        # dense PE block at the window end (its exps are all done by then,
        # and the single av PSUM accumulator pair is free).
        for j in range(NCH):
            ct_all[j] = ct_all.get(j, [])
            fillers = []
            hs_tail = []
            if j + 1 < NCH:
                ct_all[j + 1] = []
                fillers.extend(qkv_units(j + 1))
            if j >= 1:
                fillers.extend(outproj_units(j - 1))
            if HEADSTART and j + 1 < NCH:
                hs_scores, hs_tail = att_unit_lists(j + 1, 0)
                fillers.extend(hs_scores)
            main = []
            for t in range(1 if (HEADSTART and j >= 1) else 0, NMT):
                main.extend(att_main_units(j, t))
            n_units = len(main)
            n_fill = len(fillers)
            popped = 0
            for i, u in enumerate(main):
                u()
                while fillers and popped < (i + 1) * n_fill // n_units:
                    fillers.pop(0)()
                    popped += 1
            for f in fillers:
                f()
            for f in hs_tail:
                f()

        # final chunk's out-projection (pipelined against the last norms)
        for unit in outproj_tail(NCH - 1):
            unit()


_PROG = None


def _build():
    global _PROG
    if _PROG is not None:
        return _PROG
    import concourse.bacc as bacc
    import concourse.mybir as mybir
    import concourse.tile as tile

    f32 = mybir.dt.float32
    bf16 = mybir.dt.bfloat16
    nc = bacc.Bacc(
        "TRN2", target_bir_lowering=False, debug=False, enable_asserts=False
    )
    xT = nc.dram_tensor("xT", [128, NCH * ND * CH], bf16, kind="ExternalInput").ap()
    wq = nc.dram_tensor("wq", [128, ND * M], bf16, kind="ExternalInput").ap()
    wk = nc.dram_tensor("wk", [128, ND * M], bf16, kind="ExternalInput").ap()
    wv = nc.dram_tensor("wv", [128, ND * M], bf16, kind="ExternalInput").ap()
    wo = nc.dram_tensor("wo", [128, NMT * D], bf16, kind="ExternalInput").ap()
    outT = nc.dram_tensor("outT", [D, S], bf16, kind="ExternalOutput").ap()

    with tile.TileContext(nc) as tc:
        _emit(nc, tc, tile, mybir, (xT, wq, wk, wv, wo, outT))
    nc.compile()
    _PROG = nc
    return nc


def kernel(x, Wq, Wk, Wv, Wo, bo):
    global LAST_RESULT
    import os

    from concourse.bass_utils import run_bass_kernel_spmd

    x = np.asarray(x, dtype=np.float32)
    Wq = np.asarray(Wq, dtype=np.float32)
    Wk = np.asarray(Wk, dtype=np.float32)
    Wv = np.asarray(Wv, dtype=np.float32)
    Wo = np.asarray(Wo, dtype=np.float32)
    bo = np.asarray(bo, dtype=np.float32)

    nc = _build()

    import ml_dtypes

    bf = ml_dtypes.bfloat16

    def fold_w(w):
        # [(nd p), c] -> [p, (nd c)]
        ndt = w.shape[0] // 128
        return np.ascontiguousarray(
            w.reshape(ndt, 128, w.shape[1]).transpose(1, 0, 2).reshape(128, -1)
        ).astype(bf)

    in_maps = []
    for c in range(NCORE):
        b, g = c // 2, c % 2
        cols = slice(M * g, M * (g + 1))
        xt = x[b].T  # [D, S]
        # [p, (j d s)]: xf[p, j*ND*CH + d*CH + s] = xT[128d+p, CH*j+s]
        xf = (
            xt.reshape(ND, 128, NCH, CH)
            .transpose(1, 2, 0, 3)
            .reshape(128, NCH * ND * CH)
        )
        in_maps.append(
            {
                "xT": np.ascontiguousarray(xf).astype(bf),
                "wq": fold_w(Wq[:, cols]),
                "wk": fold_w(Wk[:, cols]),
                "wv": fold_w(Wv[:, cols]),
                "wo": fold_w(Wo[cols, :]),
            }
        )

    res = run_bass_kernel_spmd(
        nc,
        in_maps,
        list(range(NCORE)),
        trace=bool(os.environ.get("KERNEL_TRACE")),
        tmpdir=os.environ.get("KERNEL_TRACE_DIR") or None,
    )
    LAST_RESULT = res

    out = np.empty((B, S, D), dtype=np.float32)
    for b in range(B):
        acc = (
            res.results[2 * b]["outT"].astype(np.float32)
            + res.results[2 * b + 1]["outT"].astype(np.float32)
        )
        out[b] = acc.T + bo[None, :]
    return out
